# revision 2
# baseline (speedup 1.0000x reference)
"""Trainium2 Bass kernel for a binary-conv BasicBlock:
out = move2(prelu(move1(bn(conv3x3(sign(x+b0), scale*sign(w))) + x)))

Strategy: data-parallel over batch across 8 NeuronCores. Per core:
  - activations live as [Cin=128 partitions, n, h, w] in SBUF
  - sign(x+bias0) computed on ScalarE into a zero-padded bf16 buffer
  - conv3x3 = 9 accumulating 128x128 matmuls (one per tap) into PSUM;
    products are +-1 so bf16 matmul with f32 PSUM accumulation is exact
  - BN batch stats via bn_stats/bn_aggr per core + 128x2 AllReduce
  - scale/gamma/beta/bias1 folded into per-channel affine A*z+B on device
  - epilogue: A*z+B (ScalarE) -> +residual (VectorE) -> PReLU (ScalarE)
    -> +bias2 (VectorE) -> DMA out
"""
import numpy as np
import ml_dtypes

import concourse.bass as bass
import concourse.bacc as bacc
import concourse.tile as tile
from concourse import mybir
from concourse.bass_utils import run_bass_kernel_spmd

N_CORES = 8
B, C, H, W = 32, 128, 56, 56
NB = B // N_CORES          # images per core
HP, WP = H + 2, W + 2      # padded plane
RB = 8                     # output rows per conv block
BLKS = H // RB             # conv blocks per image
EPS = 1e-5

F32 = mybir.dt.float32
BF16 = mybir.dt.bfloat16


def _build():
    nc = bacc.Bacc("TRN2", target_bir_lowering=False, debug=False,
                   num_devices=N_CORES)

    x_d = nc.dram_tensor("x", [NB, C, H, W], F32, kind="ExternalInput")
    wsT_d = nc.dram_tensor("wsT", [C, 9, C], BF16, kind="ExternalInput")
    b0_d = nc.dram_tensor("b0", [C, 1], F32, kind="ExternalInput")
    # coef columns: 0=gamma*scale, 1=scale^2, 2=beta+bias1, 3=alpha, 4=bias2
    coef_d = nc.dram_tensor("coef", [C, 5], F32, kind="ExternalInput")
    out_d = nc.dram_tensor("out", [NB, C, H, W], F32, kind="ExternalOutput")

    with tile.TileContext(nc) as tc:
        with tc.tile_pool(name="big", bufs=1) as big, \
             tc.tile_pool(name="small", bufs=1) as small, \
             tc.tile_pool(name="psum", bufs=4, space="PSUM") as psum, \
             tc.tile_pool(name="opool", bufs=3) as opool, \
             tc.tile_pool(name="dram", bufs=1, space="DRAM") as dram:

            x_sb = big.tile([C, NB, H, W], F32)
            a_pad = big.tile([C, NB, HP, WP], BF16)
            z = big.tile([C, NB, H, W], F32)
            wsT = small.tile([C, 9, C], BF16)
            b0 = small.tile([C, 1], F32)
            coef = small.tile([C, 5], F32)
            stats = small.tile([C, NB * BLKS, 6], F32)

            nc.sync.dma_start(out=wsT[:], in_=wsT_d.ap())
            nc.sync.dma_start(out=b0[:], in_=b0_d.ap())
            nc.sync.dma_start(out=coef[:], in_=coef_d.ap())

            # zero only the padding border of a_pad
            for n in range(NB):
                nc.vector.memset(a_pad[:, n, 0, :], 0.0)
                nc.vector.memset(a_pad[:, n, HP - 1, :], 0.0)
                nc.vector.memset(a_pad[:, n, 1:HP - 1, 0:1], 0.0)
                nc.vector.memset(a_pad[:, n, 1:HP - 1, WP - 1:WP], 0.0)

            for n in range(NB):
                nc.sync.dma_start(out=x_sb[:, n], in_=x_d.ap()[n])
                nc.scalar.activation(
                    out=a_pad[:, n, 1:HP - 1, 1:WP - 1],
                    in_=x_sb[:, n],
                    func=mybir.ActivationFunctionType.Sign,
                    bias=b0[:],
                    scale=1.0,
                )

            # conv: 9 accumulating matmuls per [128, RB*W] output block
            for n in range(NB):
                for hb in range(BLKS):
                    h0 = hb * RB
                    ps = psum.tile([C, RB * W], F32)
                    for t in range(9):
                        kh, kw = t // 3, t % 3
                        nc.tensor.matmul(
                            ps[:],
                            wsT[:, t, :],
                            a_pad[:, n, h0 + kh:h0 + kh + RB, kw:kw + W],
                            start=(t == 0),
                            stop=(t == 8),
                        )
                    nc.vector.bn_stats(out=stats[:, n * BLKS + hb, :], in_=ps[:])
                    nc.scalar.activation(
                        out=z[:, n, h0:h0 + RB, :], in_=ps[:],
                        func=mybir.ActivationFunctionType.Copy,
                    )

            # local mean/var -> payload [mean, mean^2+var] -> AllReduce
            mv = small.tile([C, 2], F32)
            nc.vector.bn_aggr(out=mv[:], in_=stats[:])
            payload = small.tile([C, 2], F32)
            nc.vector.tensor_copy(out=payload[:, 0:1], in_=mv[:, 0:1])
            nc.vector.tensor_scalar(
                out=payload[:, 1:2], in0=mv[:, 0:1],
                scalar1=mv[:, 0:1], scalar2=mv[:, 1:2],
                op0=mybir.AluOpType.mult, op1=mybir.AluOpType.add,
            )

            cc_in = dram.tile([C, 2], F32)
            cc_out = dram.tile([C, 2], F32, addr_space="Shared")
            nc.sync.dma_start(out=cc_in[:], in_=payload[:])
            nc.gpsimd.collective_compute(
                "AllReduce",
                mybir.AluOpType.add,
                ins=[cc_in.opt()],
                outs=[cc_out.opt()],
                replica_groups=[list(range(N_CORES))],
            )
            g = small.tile([C, 2], F32)
            nc.sync.dma_start(out=g[:], in_=cc_out[:])

            # global coefficients: A = gs * rsqrt(s2*var + eps), B = beta1 - A*m
            neg_m = small.tile([C, 1], F32)
            q = small.tile([C, 1], F32)
            var = small.tile([C, 1], F32)
            sd = small.tile([C, 1], F32)
            rs = small.tile([C, 1], F32)
            A = small.tile([C, 1], F32)
            Bt = small.tile([C, 1], F32)
            nc.vector.tensor_scalar_mul(out=neg_m[:], in0=g[:, 0:1],
                                        scalar1=-1.0 / N_CORES)
            nc.vector.tensor_scalar_mul(out=q[:], in0=g[:, 1:2],
                                        scalar1=1.0 / N_CORES)
            # var = q - m^2 = q - neg_m*neg_m
            nc.vector.tensor_mul(out=var[:], in0=neg_m[:], in1=neg_m[:])
            nc.vector.tensor_sub(out=var[:], in0=q[:], in1=var[:])
            nc.vector.tensor_scalar(
                out=var[:], in0=var[:], scalar1=coef[:, 1:2], scalar2=EPS,
                op0=mybir.AluOpType.mult, op1=mybir.AluOpType.add,
            )
            nc.scalar.activation(out=sd[:], in_=var[:],
                                 func=mybir.ActivationFunctionType.Sqrt)
            nc.vector.reciprocal(out=rs[:], in_=sd[:])
            nc.vector.tensor_scalar_mul(out=A[:], in0=rs[:], scalar1=coef[:, 0:1])
            nc.vector.tensor_scalar(
                out=Bt[:], in0=A[:], scalar1=neg_m[:], scalar2=coef[:, 2:3],
                op0=mybir.AluOpType.mult, op1=mybir.AluOpType.add,
            )

            # epilogue, per half image
            RHALF = H // 2
            for n in range(NB):
                for half in range(2):
                    r0 = half * RHALF
                    sl = z[:, n, r0:r0 + RHALF, :]
                    # sl = A*z + x  (B folds into the Prelu pre-bias)
                    nc.vector.scalar_tensor_tensor(
                        out=sl, in0=sl, scalar=A[:],
                        in1=x_sb[:, n, r0:r0 + RHALF, :],
                        op0=mybir.AluOpType.mult, op1=mybir.AluOpType.add,
                    )
                    o = opool.tile([C, RHALF, W], F32)
                    nc.scalar.activation(
                        out=o[:], in_=sl,
                        func=mybir.ActivationFunctionType.Prelu,
                        bias=Bt[:], scale=1.0,
                        alpha=coef[:, 3:4],
                    )
                    nc.vector.tensor_scalar_add(out=o[:], in0=o[:],
                                                scalar1=coef[:, 4:5])
                    nc.sync.dma_start(out=out_d.ap()[n, :, r0:r0 + RHALF, :],
                                      in_=o[:])

    nc.compile()
    return nc


_NC_CACHE = None


def _get_nc():
    global _NC_CACHE
    if _NC_CACHE is None:
        _NC_CACHE = _build()
    return _NC_CACHE


def _make_in_maps(x, bias0, w, gamma, beta, bias1, alpha, bias2):
    x = np.asarray(x, np.float32)
    w = np.asarray(w, np.float32)
    sign_w = np.sign(w).astype(np.float32)  # [Cout, Cin, 3, 3]
    wsT = np.ascontiguousarray(
        sign_w.reshape(C, C, 9).transpose(1, 2, 0)
    ).astype(ml_dtypes.bfloat16)            # [Cin, 9, Cout]
    scale = np.abs(w).mean(axis=(1, 2, 3)).astype(np.float32)  # [Cout]

    coef = np.stack([
        np.asarray(gamma, np.float32) * scale,
        scale * scale,
        np.asarray(beta, np.float32) + np.asarray(bias1, np.float32),
        np.asarray(alpha, np.float32),
        np.asarray(bias2, np.float32),
    ], axis=1).astype(np.float32)           # [C, 5]
    b0 = np.asarray(bias0, np.float32).reshape(C, 1)

    in_maps = []
    for i in range(N_CORES):
        in_maps.append({
            "x": np.ascontiguousarray(x[i * NB:(i + 1) * NB]),
            "wsT": wsT,
            "b0": b0,
            "coef": coef,
        })
    return in_maps


def kernel(x, bias0, w, gamma, beta, bias1, alpha, bias2):
    nc = _get_nc()
    in_maps = _make_in_maps(x, bias0, w, gamma, beta, bias1, alpha, bias2)
    res = run_bass_kernel_spmd(nc, in_maps, list(range(N_CORES)))
    out = np.concatenate([res.results[i]["out"] for i in range(N_CORES)], axis=0)
    return out.astype(np.float32)


# revision 4
# speedup vs baseline: 347.1317x; 347.1317x over previous
"""Trainium2 Bass kernel for a binary-conv BasicBlock:
out = move2(prelu(move1(bn(conv3x3(sign(x+b0), scale*sign(w))) + x)))

Strategy: data-parallel over batch across 8 NeuronCores. Per core:
  - activations live as [Cin=128 partitions, n, h, w] in SBUF
  - sign(x+bias0) computed on ScalarE into a zero-padded bf16 buffer
  - conv3x3 = 9 accumulating 128x128 matmuls (one per tap) into PSUM;
    products are +-1 so bf16 matmul with f32 PSUM accumulation is exact
  - BN batch stats via bn_stats/bn_aggr per core + 128x2 AllReduce
  - scale/gamma/beta/bias1 folded into per-channel affine A*z+B on device
  - epilogue: A*z+B (ScalarE) -> +residual (VectorE) -> PReLU (ScalarE)
    -> +bias2 (VectorE) -> DMA out
"""
import numpy as np
import ml_dtypes

import concourse.bass as bass
import concourse.bacc as bacc
import concourse.tile as tile
from concourse import mybir
from concourse.bass_utils import run_bass_kernel_spmd

N_CORES = 8
B, C, H, W = 32, 128, 56, 56
NB = B // N_CORES          # images per core
HP, WP = H + 2, W + 2      # padded plane
RB = 8                     # output rows per conv block
BLKS = H // RB             # conv blocks per image
EPS = 1e-5

F32 = mybir.dt.float32
BF16 = mybir.dt.bfloat16


def _build(reps=1):
    nc = bacc.Bacc("TRN2", target_bir_lowering=False, debug=False,
                   num_devices=N_CORES)

    x_d = nc.dram_tensor("x", [NB, C, H, W], F32, kind="ExternalInput")
    wsT_d = nc.dram_tensor("wsT", [C, 9, C], BF16, kind="ExternalInput")
    b0_d = nc.dram_tensor("b0", [C, 1], F32, kind="ExternalInput")
    # coef columns: 0=gamma*scale, 1=scale^2, 2=beta+bias1, 3=alpha, 4=bias2
    coef_d = nc.dram_tensor("coef", [C, 5], F32, kind="ExternalInput")
    out_d = nc.dram_tensor("out", [NB, C, H, W], F32, kind="ExternalOutput")

    with tile.TileContext(nc) as tc:
        with tc.tile_pool(name="big", bufs=1) as big, \
             tc.tile_pool(name="small", bufs=1) as small, \
             tc.tile_pool(name="psum", bufs=4, space="PSUM") as psum, \
             tc.tile_pool(name="opool", bufs=3) as opool, \
             tc.tile_pool(name="dram", bufs=1, space="DRAM") as dram:
            for _ in range(reps):
                _emit_iter(nc, tc, big, small, psum, opool, dram,
                           x_d, wsT_d, b0_d, coef_d, out_d)

    nc.compile()
    return nc


def _emit_iter(nc, tc, big, small, psum, opool, dram,
               x_d, wsT_d, b0_d, coef_d, out_d):
    if True:
        if True:
            x_sb = big.tile([C, NB, H, W], F32)
            a_pad = big.tile([C, NB, HP, WP], BF16)
            z = big.tile([C, NB, H, W], F32)
            wsT = small.tile([C, 9, C], BF16)
            b0 = small.tile([C, 1], F32)
            coef = small.tile([C, 5], F32)
            stats = small.tile([C, NB * BLKS, 6], F32)

            nc.sync.dma_start(out=wsT[:], in_=wsT_d.ap())
            nc.sync.dma_start(out=b0[:], in_=b0_d.ap())
            nc.sync.dma_start(out=coef[:], in_=coef_d.ap())

            # zero only the padding border of a_pad
            for n in range(NB):
                nc.vector.memset(a_pad[:, n, 0, :], 0.0)
                nc.vector.memset(a_pad[:, n, HP - 1, :], 0.0)
                nc.vector.memset(a_pad[:, n, 1:HP - 1, 0:1], 0.0)
                nc.vector.memset(a_pad[:, n, 1:HP - 1, WP - 1:WP], 0.0)

            for n in range(NB):
                nc.sync.dma_start(out=x_sb[:, n], in_=x_d.ap()[n])
                nc.scalar.activation(
                    out=a_pad[:, n, 1:HP - 1, 1:WP - 1],
                    in_=x_sb[:, n],
                    func=mybir.ActivationFunctionType.Sign,
                    bias=b0[:],
                    scale=1.0,
                )

            # conv: 9 accumulating matmuls per [128, RB*W] output block
            for n in range(NB):
                for hb in range(BLKS):
                    h0 = hb * RB
                    ps = psum.tile([C, RB * W], F32)
                    for t in range(9):
                        kh, kw = t // 3, t % 3
                        nc.tensor.matmul(
                            ps[:],
                            wsT[:, t, :],
                            a_pad[:, n, h0 + kh:h0 + kh + RB, kw:kw + W],
                            start=(t == 0),
                            stop=(t == 8),
                        )
                    nc.vector.bn_stats(out=stats[:, n * BLKS + hb, :], in_=ps[:])
                    nc.scalar.activation(
                        out=z[:, n, h0:h0 + RB, :], in_=ps[:],
                        func=mybir.ActivationFunctionType.Copy,
                    )

            # local mean/var -> payload [mean, mean^2+var] -> AllReduce
            mv = small.tile([C, 2], F32)
            nc.vector.bn_aggr(out=mv[:], in_=stats[:])
            payload = small.tile([C, 2], F32)
            nc.vector.tensor_copy(out=payload[:, 0:1], in_=mv[:, 0:1])
            nc.vector.tensor_scalar(
                out=payload[:, 1:2], in0=mv[:, 0:1],
                scalar1=mv[:, 0:1], scalar2=mv[:, 1:2],
                op0=mybir.AluOpType.mult, op1=mybir.AluOpType.add,
            )

            cc_in = dram.tile([C, 2], F32)
            cc_out = dram.tile([C, 2], F32, addr_space="Shared")
            nc.sync.dma_start(out=cc_in[:], in_=payload[:])
            nc.gpsimd.collective_compute(
                "AllReduce",
                mybir.AluOpType.add,
                ins=[cc_in.opt()],
                outs=[cc_out.opt()],
                replica_groups=[list(range(N_CORES))],
            )
            g = small.tile([C, 2], F32)
            nc.sync.dma_start(out=g[:], in_=cc_out[:])

            # global coefficients: A = gs * rsqrt(s2*var + eps), B = beta1 - A*m
            neg_m = small.tile([C, 1], F32)
            q = small.tile([C, 1], F32)
            var = small.tile([C, 1], F32)
            sd = small.tile([C, 1], F32)
            rs = small.tile([C, 1], F32)
            A = small.tile([C, 1], F32)
            Bt = small.tile([C, 1], F32)
            nc.vector.tensor_scalar_mul(out=neg_m[:], in0=g[:, 0:1],
                                        scalar1=-1.0 / N_CORES)
            nc.vector.tensor_scalar_mul(out=q[:], in0=g[:, 1:2],
                                        scalar1=1.0 / N_CORES)
            # var = q - m^2 = q - neg_m*neg_m
            nc.vector.tensor_mul(out=var[:], in0=neg_m[:], in1=neg_m[:])
            nc.vector.tensor_sub(out=var[:], in0=q[:], in1=var[:])
            nc.vector.tensor_scalar(
                out=var[:], in0=var[:], scalar1=coef[:, 1:2], scalar2=EPS,
                op0=mybir.AluOpType.mult, op1=mybir.AluOpType.add,
            )
            nc.scalar.activation(out=sd[:], in_=var[:],
                                 func=mybir.ActivationFunctionType.Sqrt)
            nc.vector.reciprocal(out=rs[:], in_=sd[:])
            nc.vector.tensor_scalar_mul(out=A[:], in0=rs[:], scalar1=coef[:, 0:1])
            nc.vector.tensor_scalar(
                out=Bt[:], in0=A[:], scalar1=neg_m[:], scalar2=coef[:, 2:3],
                op0=mybir.AluOpType.mult, op1=mybir.AluOpType.add,
            )

            # epilogue, per half image
            RHALF = H // 2
            for n in range(NB):
                for half in range(2):
                    r0 = half * RHALF
                    sl = z[:, n, r0:r0 + RHALF, :]
                    # sl = A*z + x  (B folds into the Prelu pre-bias)
                    nc.vector.scalar_tensor_tensor(
                        out=sl, in0=sl, scalar=A[:],
                        in1=x_sb[:, n, r0:r0 + RHALF, :],
                        op0=mybir.AluOpType.mult, op1=mybir.AluOpType.add,
                    )
                    o = opool.tile([C, RHALF, W], F32)
                    nc.scalar.activation(
                        out=o[:], in_=sl,
                        func=mybir.ActivationFunctionType.Prelu,
                        bias=Bt[:], scale=1.0,
                        alpha=coef[:, 3:4],
                    )
                    nc.vector.tensor_scalar_add(out=o[:], in0=o[:],
                                                scalar1=coef[:, 4:5])
                    nc.sync.dma_start(out=out_d.ap()[n, :, r0:r0 + RHALF, :],
                                      in_=o[:])


_NC_CACHE = {}


def _get_nc(reps=1):
    if reps not in _NC_CACHE:
        _NC_CACHE[reps] = _build(reps)
    return _NC_CACHE[reps]


def _make_in_maps(x, bias0, w, gamma, beta, bias1, alpha, bias2):
    x = np.asarray(x, np.float32)
    w = np.asarray(w, np.float32)
    sign_w = np.sign(w).astype(np.float32)  # [Cout, Cin, 3, 3]
    wsT = np.ascontiguousarray(
        sign_w.reshape(C, C, 9).transpose(1, 2, 0)
    ).astype(ml_dtypes.bfloat16)            # [Cin, 9, Cout]
    scale = np.abs(w).mean(axis=(1, 2, 3)).astype(np.float32)  # [Cout]

    coef = np.stack([
        np.asarray(gamma, np.float32) * scale,
        scale * scale,
        np.asarray(beta, np.float32) + np.asarray(bias1, np.float32),
        np.asarray(alpha, np.float32),
        np.asarray(bias2, np.float32),
    ], axis=1).astype(np.float32)           # [C, 5]
    b0 = np.asarray(bias0, np.float32).reshape(C, 1)

    in_maps = []
    for i in range(N_CORES):
        in_maps.append({
            "x": np.ascontiguousarray(x[i * NB:(i + 1) * NB]),
            "wsT": wsT,
            "b0": b0,
            "coef": coef,
        })
    return in_maps


def kernel(x, bias0, w, gamma, beta, bias1, alpha, bias2):
    nc = _get_nc()
    in_maps = _make_in_maps(x, bias0, w, gamma, beta, bias1, alpha, bias2)
    res = run_bass_kernel_spmd(nc, in_maps, list(range(N_CORES)))
    out = np.concatenate([res.results[i]["out"] for i in range(N_CORES)], axis=0)
    return out.astype(np.float32)
